# revision 1
# baseline (speedup 1.0000x reference)
"""Batched attention (no 1/sqrt(d) scaling) for Trainium2, 8 NeuronCores.

Problem: q,k,v [128, 1024, 64] fp32 ->
    out = softmax(q @ k^T, axis=-1) @ v   per batch.

Sharding: batch dim 128 split as 16 batches per core (data parallel, no
communication).

Per-core algorithm (per batch b), designed so the Activation engine (the only
engine that can do exp, at 1 elem/lane/cycle) is the sole bottleneck:

  1. Load q,k as f32r [128, 8, 64] tiles, q's first half first (MM1(0)
     needs ALL of qT but only kT slice 0, so q's arrival is the latest
     startup dependency); PE-transpose (f32r: 1.5 cyc/row) into qT,kT
     [64, 1024] (d on partitions), staged through PSUM with DVE PSUM->SBUF
     copies (GPSIMD cannot access PSUM).
  2. Load v, convert to bf16 va [128, 8, 65] with a ones column (col 64).
  3. For each t-tile (128 rows of k): scoresT[t, s] = kT_tile^T @ qT in PSUM
     [128, 1024] (f32r, 1 cyc/col); exp via ACT straight into bf16 expT
     (no max subtraction: |scores| <= ~45, exp fits fp32/bf16 range).
  4. MM2 with exp as the STATIONARY operand: for each s-tile,
     acc[s, 0:65] += expT[:, s-tile]^T @ va[t]  -- output lands directly in
     [s, d] layout (plus denominator in col 64), bf16 at 1 cyc/col with only
     65 moving columns.  No attention transposes, no output transposes.
     The 8 accumulators share 2 PSUM banks, and matmul start=True resets a
     WHOLE bank, so the accumulator is DVE-memset once per batch and every
     matmul runs in accumulate mode.
  5. Normalize on DVE: one PSUM->SBUF copy releases the accumulator, then
     out[s, 0:64] = acc[s, 0:64] * recip(acc[s, 64]) per s-tile; DMA out.

Pipelining: 8 expT buffers keep ACT (the bottleneck at ~1.04us per
[128,1024] exp) busy across the norm-copy/memset accumulator handoff; batch
b's last two MM2 tile-steps are emitted after batch b+1's input transposes;
8 dummy transposes at stream head warm the PE p-state while the first DMAs
are in flight; the final batch normalizes and stores per psum-bank half.

HW facts learned the hard way (verified on trn2):
  - GPSIMD cannot access PSUM, and its tensor_scalar AP-scalar path
    miscomputes; DVE reciprocal needs free size 1... and its output written
    to adjacent columns of a shared tile gets clobbered (scratch overwrite).
  - matmul start=True zeroes the entire target PSUM bank.
  - The PE dispatch queue (depth 32) executes ready instructions out of
    program order, so correctness may not rely on same-engine ordering.
  - f32r matmuls with <256-col outputs violate s3d3_mm_fp32r_restrictions;
    matmul outputs are capped at 512 elements (s3d3_mm_num_elements).
"""

import os
import sys
from contextlib import ExitStack

sys.path.insert(0, "/opt/trn_rl_repo")

import numpy as np

import concourse.bass as bass
import concourse.tile as tile
from concourse import mybir
from concourse.bass_utils import run_bass_kernel_spmd
from concourse.masks import make_identity

# ---------------------------------------------------------------------------
# Workaround: this walrus build allows only ONE semaphore wait per
# instruction (TPB_EVENTS has a single wait slot).  The stock Tile epilogue
# attaches every outstanding sem wait to a single SP Drain, which fails
# codegen with "Too many sync wait commands".  Split the waits across one
# Drain per semaphore instead (SP executes them sequentially, semantics are
# identical).
# ---------------------------------------------------------------------------
import bass_rust
from concourse.vector_clock import ScopedClock


def _split_wait_drain_and_barrier(self, tick_clock, wait_clock):
    nc = self.nc
    drain_inst = nc.sync.drain()
    wait_clock.add_sem_waits(
        drain_inst.ins, ScopedClock({None: tick_clock.global_clock})
    )
    ins = drain_inst.ins
    si = ins.sync_info
    if si is not None and si.on_wait and len(si.on_wait) > 1:
        waits = list(si.on_wait)
        si.on_wait = waits[:1]
        for w in waits[1:]:
            extra = nc.sync.drain()
            extra_ins = extra.ins
            if extra_ins.sync_info is None:
                extra_ins.sync_info = bass_rust.SyncInfo(on_wait=[w], on_update=[])
            else:
                extra_ins.sync_info.on_wait = [w]

    nc.all_engine_barrier()
    assert self.sems is not None
    popped = nc._tile_sem_poison_stack.pop()
    assert popped is self._sem_poison
    nc.clear_and_free_semaphores(list(self.sems.allocated().values()))
    nc.all_engine_barrier()


tile.TileContext._drain_and_barrier = _split_wait_drain_and_barrier


def _legalize_single_wait(nc):
    """Rewrite every instruction carrying N>1 sem waits into N-1 single-wait
    NoOps (same engine, inserted just before it) + the instruction keeping one
    wait.  Same-engine execution is in-order, so semantics are preserved."""
    fn = nc.m.functions[0]
    for blk in fn.blocks:
        insts = blk.instructions
        if not any(
            i.sync_info is not None and i.sync_info.on_wait and len(i.sync_info.on_wait) > 1
            for i in insts
        ):
            continue
        out = []
        for inst in insts:
            si = inst.sync_info
            if si is not None and si.on_wait and len(si.on_wait) > 1:
                waits = list(si.on_wait)
                for w in waits[:-1]:
                    out.append(
                        mybir.InstNoOp(
                            name=nc.get_next_instruction_name(),
                            engine=inst.engine,
                            sync_info=mybir.SyncInfo(on_wait=[w], on_update=[]),
                            bass_nofuse=True,
                        )
                    )
                si.on_wait = waits[-1:]
            out.append(inst)
        blk.instructions = out


# ---------------------------------------------------------------------------

N_CORES = 8
B, S, D = 128, 1024, 64
B_LOC = B // N_CORES  # batches per core
NT = S // 128  # 128-row tiles per sequence

F32 = mybir.dt.float32
F32R = mybir.dt.float32r
BF16 = mybir.dt.bfloat16

# Transpose-path dtype: f32r transposes are 1.5 cyc/row vs 2.0 for f32.
TR_DT = F32R if os.environ.get("ATTN_TR", "f32r") == "f32r" else F32
# exp/MM2 dtype: bf16 (1 cyc/col MM2) or f32r (better accuracy, 4 cyc/col).
EXP_DT = F32R if os.environ.get("ATTN_EXP_F32") == "1" else BF16


def _attention_body(tc, o, q, k, v, reps=1):
    nc = tc.nc
    with ExitStack() as ctx:
        singles = ctx.enter_context(tc.tile_pool(name="singles", bufs=1))
        # Dependency-free warmup source: lets the PE p-state dummies start
        # ~1us before make_identity finishes.
        dsrc = singles.tile([128, 128], F32)
        nc.vector.memset(dsrc, 1.0)
        # GPSIMD memset can't write f32r directly; build in f32, copy once.
        ident_f32 = singles.tile([128, 128], F32)
        make_identity(nc, ident_f32)
        ident = singles.tile([128, 128], TR_DT)
        nc.vector.tensor_copy(ident, ident_f32)
        ones1 = singles.tile([128, NT], F32)
        nc.vector.memset(ones1, 1.0)

        qk_pool = ctx.enter_context(tc.tile_pool(name="qk", bufs=2))
        v_pool = ctx.enter_context(tc.tile_pool(name="vp", bufs=2))
        va_pool = ctx.enter_context(tc.tile_pool(name="vap", bufs=2))
        qkt_pool = ctx.enter_context(tc.tile_pool(name="qkt", bufs=2))
        exp_pool = ctx.enter_context(tc.tile_pool(name="expp", bufs=8))
        out_pool = ctx.enter_context(tc.tile_pool(name="outp", bufs=2))
        acc_pool = ctx.enter_context(tc.tile_pool(name="accp", bufs=2))
        r_pool = ctx.enter_context(tc.tile_pool(name="rp", bufs=4))
        # PSUM budget (8 banks): transpose staging 2x[64,512] = 2, scores
        # 2x[128,1024] = 4, output accumulator 1x[128,8,128] = 2.
        ps_tr_pool = ctx.enter_context(tc.tile_pool(name="ps_tr", bufs=2, space="PSUM"))
        ps_sc = ctx.enter_context(tc.tile_pool(name="ps_sc", bufs=2, space="PSUM"))
        ps_acc = ctx.enter_context(tc.tile_pool(name="ps_acc", bufs=1, space="PSUM"))

        def emit_mm2(ps_o, va, expT, n):
            # exp as stationary: acc[s, 0:65] += expT[:, s-tile]^T @ va[t].
            # start=True resets the ENTIRE psum bank (verified on HW), which
            # would wipe the other 3 s-slots sharing the bank -- so the bank
            # is zeroed once by DVE memset and every matmul accumulates.
            if os.environ.get("ATTN_SKIP_MM2") == "1":
                return
            for sidx in range(NT):
                nc.tensor.matmul(
                    ps_o[:, sidx, 0 : D + 1],
                    expT[:, bass.ts(sidx, 128)],
                    va[:, n, :],
                    start=False,
                    stop=False,
                    skip_group_check=True,
                )

        def emit_norm(ps_o, ob):
            # One fast PSUM->SBUF copy releases the accumulator for the next
            # batch's MM2s; reciprocal+scale then run off the critical chain.
            accs = acc_pool.tile([128, NT, D + 1], F32, tag="accs")
            nc.vector.tensor_copy(accs, ps_o[:, :, 0 : D + 1])
            # DVE reciprocal with free size > 1 miscomputes on HW (only the
            # last element of each 4-group is right) -- keep it [128, 1].
            ou = out_pool.tile([128, NT, D], F32, tag="ou")
            for sidx in range(NT):
                rec = r_pool.tile([128, 1], F32, tag="rec")
                nc.vector.reciprocal(rec, accs[:, sidx, D : D + 1])
                nc.vector.tensor_scalar_mul(ou[:, sidx, :], accs[:, sidx, 0:D], rec)
            nc.sync.dma_start(out=ob, in_=ou)

        def emit_all():
            # PE p-state warmup: the tensor engine ramps to full clock only
            # after ~3us of continuous execution.  Keep it busy with dummy
            # transposes (into the first batch's accumulator slot, which is
            # memset later anyway) while the first DMAs are in flight, so the
            # real transposes run at full clock.
            warm = ps_acc.tile([128, NT, 128], F32, tag="acc")
            for w in range(8):
                nc.tensor.transpose(warm[:, w % NT, 0:128], dsrc, dsrc)

            # pending per-batch tail work carried into the next batch's
            # emission: (ps_o, va, expT6, expT7, ob) -- MM2 for t-tiles 6,7 +
            # normalize.
            pending = None

            for b in range(B_LOC):
                qb = q[b].rearrange("(n p) d -> p n d", p=128)
                kb = k[b].rearrange("(n p) d -> p n d", p=128)
                vb = v[b].rearrange("(n p) d -> p n d", p=128)
                ob = o[b].rearrange("(n p) d -> p n d", p=128)

                # Split k/q loads into halves so the first transpose group starts
                # as soon as the first 128KB lands; k's first half goes first
                # (MM1(0) needs kT slice 0 + the FULL qT).
                # q first (MM1(0) needs ALL of qT but only kT slice 0, so
                # q's arrival is the latest dependency).  Halve the transfers
                # only for batch 0, where it shortens the pipeline-fill chain;
                # later batches prefetch far ahead, so fewer/larger transfers
                # mean less DMA-path overhead (matters in contended windows).
                qn = qk_pool.tile([128, NT, D], TR_DT, tag="qn")
                kn = qk_pool.tile([128, NT, D], TR_DT, tag="kn")
                if b == 0:
                    nc.sync.dma_start(
                        out=qn[:, 0:4, :], in_=qb[:, 0:4, :].bitcast(TR_DT)
                    )
                    nc.sync.dma_start(
                        out=kn[:, 0:4, :], in_=kb[:, 0:4, :].bitcast(TR_DT)
                    )
                    nc.sync.dma_start(
                        out=qn[:, 4:NT, :], in_=qb[:, 4:NT, :].bitcast(TR_DT)
                    )
                    nc.sync.dma_start(
                        out=kn[:, 4:NT, :], in_=kb[:, 4:NT, :].bitcast(TR_DT)
                    )
                else:
                    nc.sync.dma_start(out=qn, in_=qb.bitcast(TR_DT))
                    nc.sync.dma_start(out=kn, in_=kb.bitcast(TR_DT))
                vn = v_pool.tile([128, NT, D], F32, tag="vn")
                nc.sync.dma_start(out=vn, in_=vb)

                # qT/kT [64, 1024]: PE transposes of the 8 [128, 64] subtiles,
                # staged 4-at-a-time through PSUM.  Group order k0,q0,q1,k1 so
                # MM1(0)'s deps (kT slice 0, full qT) resolve earliest.  GPSIMD
                # cannot read PSUM, so the staging copies go to DVE.
                qT = qkt_pool.tile([D, S], F32R, tag="qT")
                kT = qkt_pool.tile([D, S], F32R, tag="kT")

                def emit_tr(srcT, dstT, j, eng=None):
                    ps_tr = ps_tr_pool.tile([D, 512], TR_DT, tag="tr")
                    for i in range(4):
                        n = j * 4 + i
                        nc.tensor.transpose(
                            ps_tr[:, bass.ts(i, 128)], srcT[:, n, :], ident
                        )
                    if eng == "act":
                        nc.scalar.copy(dstT[:, bass.ts(j, 512)], ps_tr)
                    else:
                        nc.vector.tensor_copy(dstT[:, bass.ts(j, 512)], ps_tr)

                # batch 0: ACT is idle pre-stream, so it takes the k-g0
                # staging copy while DVE does the two q copies in parallel
                emit_tr(qn, qT, 0)
                emit_tr(kn, kT, 0, eng="act" if b == 0 else None)
                emit_tr(qn, qT, 1)
                # k subtiles 4-7 aren't needed until MM1(4); for batch 0 defer
                # their transposes past the first MM1s so exp(0,1) isn't
                # queued behind them on the PE.
                if b != 0:
                    emit_tr(kn, kT, 1)

                # v with ones column (col 64 accumulates the softmax
                # denominator through MM2).  bf16 needs a DVE conversion;
                # f32r is a bitcast of the f32 data.
                va = va_pool.tile([128, NT, D + 1], EXP_DT, tag="va")
                if EXP_DT is BF16:
                    nc.vector.tensor_copy(va[:, :, 0:D], vn)
                    nc.vector.memset(va[:, :, D], 1.0)
                else:
                    nc.vector.tensor_copy(va[:, :, 0:D], vn.bitcast(F32R))
                    nc.vector.tensor_copy(va[:, :, D], ones1)

                def emit_mm1(ps_s, n):
                    lhsT = kT[:, bass.ts(n, 128)]
                    # (single [128,1024]-out matmul fails the ISA 512-element
                    # cap: s3d3_mm_num_elements)
                    for h in range(2):
                        nc.tensor.matmul(
                            ps_s[:, bass.ts(h, 512)],
                            lhsT,
                            qT[:, bass.ts(h, 512)],
                            start=True,
                            stop=True,
                        )

                # MM1(b, 0) can run during exp(b-1, 7): its score slot only needs
                # exp(b-1, 6) done.  Emit it BEFORE the previous batch's pending
                # MM2(6,7) (which wait on exp(b-1, 7)) so ACT's first exp of this
                # batch starts right after the previous batch's last one.
                ps_s0 = ps_sc.tile([128, S], F32, tag="sc")
                emit_mm1(ps_s0, 0)

                # Previous batch's tail: its MM2(6,7) PE work runs while this
                # batch's staging copies complete, and its DVE normalize overlaps
                # this batch's first MM1/exp.
                if pending is not None:
                    p_ps_o, p_va, p_e6, p_e7, p_ob = pending
                    emit_mm2(p_ps_o, p_va, p_e6, NT - 2)
                    emit_mm2(p_ps_o, p_va, p_e7, NT - 1)
                    emit_norm(p_ps_o, p_ob)

                # Main loop over t-tiles: scoresT -> exp(bf16) -> MM2 accumulate.
                ps_o = ps_acc.tile([128, NT, 128], F32, tag="acc")
                nc.vector.memset(ps_o[:, :, 0 : D + 1], 0.0)
                expTs = []
                for n in range(NT):
                    if n == 0:
                        ps_s = ps_s0
                    else:
                        if b == 0 and n == 2:
                            # deferred k[4:8] transposes: needed from MM1(4),
                            # emitted here so MM1(1..3) aren't queued behind
                            # them on the PE
                            emit_tr(kn, kT, 1)
                        ps_s = ps_sc.tile([128, S], F32, tag="sc")
                        emit_mm1(ps_s, n)
                    expT = exp_pool.tile([128, S], EXP_DT, tag="expT")
                    if os.environ.get("ATTN_SKIP_EXP") == "1":
                        nc.vector.memset(expT[:, 0:1], 1.0)
                    else:
                        nc.scalar.activation(
                            expT, ps_s, mybir.ActivationFunctionType.Exp
                        )
                    expTs.append(expT)
                    # Keep PE two MM1 tiles ahead of MM2 so ACT never starves.
                    if n >= 2:
                        emit_mm2(ps_o, va, expTs[n - 2], n - 2)
                pending = (ps_o, va, expTs[NT - 2], expTs[NT - 1], ob)

            # Final batch tail, split per psum bank (s-tiles 0-3 / 4-7) so the
            # first half normalizes and DMAs while the second half's MM2s finish.
            p_ps_o, p_va, p_e6, p_e7, p_ob = pending
            emit_mm2(p_ps_o, p_va, p_e6, NT - 2)
            for half in range(2):
                ss = range(half * (NT // 2), (half + 1) * (NT // 2))
                for sidx in ss:
                    nc.tensor.matmul(
                        p_ps_o[:, sidx, 0 : D + 1],
                        p_e7[:, bass.ts(sidx, 128)],
                        p_va[:, NT - 1, :],
                        start=False,
                        stop=False,
                        skip_group_check=True,
                    )
                accs = acc_pool.tile([128, NT // 2, D + 1], F32, tag=f"acc_h{half}")
                nc.vector.tensor_copy(accs, p_ps_o[:, ss.start : ss.stop, 0 : D + 1])
                ou = out_pool.tile([128, NT // 2, D], F32, tag=f"ou_h{half}")
                for i, sidx in enumerate(ss):
                    rec = r_pool.tile([128, 1], F32, tag="rec")
                    nc.vector.reciprocal(rec, accs[:, i, D : D + 1])
                    nc.vector.tensor_scalar_mul(ou[:, i, :], accs[:, i, 0:D], rec)
                nc.sync.dma_start(out=p_ob[:, ss.start : ss.stop, :], in_=ou)

        if reps <= 1:
            emit_all()
        else:
            with tc.For_i(0, reps, 1):
                emit_all()




def build_nc(b_loc=B_LOC, legalize=True, reps=1):
    nc = bass.Bass("TRN2", target_bir_lowering=False, debug=False)
    q = nc.dram_tensor("q", [b_loc, S, D], F32, kind="ExternalInput").ap()
    k = nc.dram_tensor("k", [b_loc, S, D], F32, kind="ExternalInput").ap()
    v = nc.dram_tensor("v", [b_loc, S, D], F32, kind="ExternalInput").ap()
    o = nc.dram_tensor("out", [b_loc, S, D], F32, kind="ExternalOutput").ap()

    saved = globals()["B_LOC"]
    globals()["B_LOC"] = b_loc
    try:
        with tile.TileContext(nc) as tc:
            _attention_body(tc, o, q, k, v, reps=reps)
        if legalize:
            _legalize_single_wait(nc)
    finally:
        globals()["B_LOC"] = saved
    return nc


LAST_RESULTS = None
LAST_RUN_WALL_S = None
_NC_CACHE = {}


def kernel(q, k, v):
    import time as _time

    q = np.ascontiguousarray(np.asarray(q, dtype=np.float32))
    k = np.ascontiguousarray(np.asarray(k, dtype=np.float32))
    v = np.ascontiguousarray(np.asarray(v, dtype=np.float32))
    assert q.shape == (B, S, D), q.shape

    if "nc" not in _NC_CACHE:
        _NC_CACHE["nc"] = build_nc()
    nc = _NC_CACHE["nc"]
    in_maps = []
    for c in range(N_CORES):
        sl = slice(c * B_LOC, (c + 1) * B_LOC)
        in_maps.append({"q": q[sl], "k": k[sl], "v": v[sl]})

    t0 = _time.time()
    res = run_bass_kernel_spmd(nc, in_maps, list(range(N_CORES)))
    global LAST_RESULTS, LAST_RUN_WALL_S
    LAST_RUN_WALL_S = _time.time() - t0
    LAST_RESULTS = res
    out = np.concatenate([res.results[c]["out"] for c in range(N_CORES)], axis=0)
    return out

